# revision 1
# baseline (speedup 1.0000x reference)
"""Cross-attention (GQA) Trainium2 Bass kernel.

Problem: B=2, Tq=Tkv=2048, D_MODEL=1024, 16 query heads / 4 kv heads,
head_dim=64.  Sharded over 8 NeuronCores as batch(2) x kv-group(4); each
core computes 4 query heads + its single kv head and a partial output
projection (Wo row-split by head group); partials are summed on host.

On-chip dataflow keeps activations "transposed" (feature dim on SBUF
partitions) end-to-end so that scores, softmax and P@V need no on-chip
transposes of large tensors:

  A: qT[e,t] = WqT.T @ xqT,  kvT = WkvT.T @ xcT        (fp32r, N=512)
     v[tk,dv] via PE-transpose of vT tiles
  B: ST[tk,tq] = kT.T @ qT_h ; two heads packed in the PE array via
     row-groups (K=64 each, h_even rows 0-63, h_odd rows 64-127)
  C: P = exp(ST/8)  on ScalarE, PSUM->SBUF, 1024-wide instructions
  D: outT'[dv+sum,tq] = [v|1].T @ P ; the ones-column matmul is
     col-packed into a spare PE column-group => denominators come out
     of the same pass.  h_odd heads are placed at partitions 64..127.
  E: yT += WoT_pair.T @ outT_norm (K=128: two heads stacked)
"""

import os
import sys

import numpy as np

for _p in ("/opt/trn_rl_repo",):
    if _p not in sys.path and os.path.isdir(_p):
        sys.path.insert(0, _p)

import concourse.bass as bass
import concourse.bacc as bacc
import concourse.mybir as mybir
from concourse.tile import TileContext

# ---------------------------------------------------------------- problem dims
B = 2
TQ = 2048
TKV = 2048
D_MODEL = 1024
N_HEADS = 16
N_KV_HEADS = 4
HEAD_DIM = 64
N_CORES = 8
GROUPS = N_KV_HEADS  # kv groups = 4
HEADS_PER_DEV = N_HEADS // GROUPS  # 4
DQ = HEADS_PER_DEV * HEAD_DIM  # 256
DKV = 2 * HEAD_DIM  # 128 (k rows + v rows stacked)
SCALE = 1.0 / float(np.sqrt(HEAD_DIM))

P = 128
FREE = 512  # matmul moving-operand chunk
BLK = 1024  # tq block width (exp instruction width)

F32 = mybir.dt.float32
F32R = mybir.dt.float32r
F16 = mybir.dt.float16


def build_bass():
    nc = bacc.Bacc()

    xq = nc.declare_dram_parameter("xqT", [D_MODEL, TQ], F16, isOutput=False)
    xc = nc.declare_dram_parameter("xcT", [D_MODEL, TKV], F16, isOutput=False)
    wq = nc.declare_dram_parameter("wqT", [D_MODEL, DQ], F16, isOutput=False)
    wkv = nc.declare_dram_parameter("wkvT", [D_MODEL, DKV], F16, isOutput=False)
    wo = nc.declare_dram_parameter("woT", [DQ, D_MODEL], F16, isOutput=False)
    cid = nc.declare_dram_parameter("cid", [P, P + 64], F16, isOutput=False)
    yt = nc.declare_dram_parameter("yT", [D_MODEL, TQ], F32, isOutput=True)

    DT = D_MODEL // P  # 8 d-tiles
    ET = DQ // P  # 2 e-tiles (query head pairs)
    NCH = TQ // FREE  # 4 chunks of 512
    NTK = TKV // P  # 16 tk tiles
    NBLK = TQ // BLK  # 2 tq blocks
    JPB = BLK // FREE  # 2 free-chunks per block
    MT = D_MODEL // P  # 8 output m-tiles

    with TileContext(nc) as tc:
        with (
            tc.tile_pool(name="consts", bufs=1) as consts,
            tc.tile_pool(name="xch", bufs=3) as xpool,
            tc.tile_pool(name="pt", bufs=6) as ptpool,
            tc.tile_pool(name="nrm", bufs=2) as nrmpool,
            tc.tile_pool(name="yout", bufs=3) as ypool,
            tc.tile_pool(name="psA", bufs=2, space="PSUM") as psA,
            tc.tile_pool(name="psB", bufs=2, space="PSUM") as psB,
        ):
            # ---------------- constants / persistent tiles
            ident = consts.tile([P, P + 64], F16, tag="ident")
            nc.sync.dma_start(ident, cid[:])
            ones = ident[:, P : P + 64]

            wq_sb = consts.tile([P, DT, DQ], F16, tag="wq")
            nc.sync.dma_start(wq_sb, wq.rearrange("(i p) e -> p i e", p=P))
            wkv_sb = consts.tile([P, DT, DKV], F16, tag="wkv")
            nc.sync.dma_start(wkv_sb, wkv.rearrange("(i p) e -> p i e", p=P))
            wo_sb = consts.tile([P, ET, D_MODEL], F16, tag="wo")
            nc.sync.dma_start(wo_sb, wo.rearrange("(i p) m -> p i m", p=P))

            qt = consts.tile([P, ET, TQ], F16, tag="qt")  # qT: heads 2/tile
            kv = consts.tile([P, TKV], F16, tag="kv")  # rows 0-63 kT, 64-127 vT
            k2 = consts.tile([P, TKV], F16, tag="k2")  # rows 64-127 = kT copy
            vp = consts.tile([P, NTK, P], F16, tag="vp")  # [v | ones]
            vp2 = consts.tile([P, NTK, P], F16, tag="vp2")  # [ones | v]
            outs = consts.tile([P, ET, TQ], F16, tag="outs")  # normalized outT

            # ---------------- stage A: projections (weights stationary)
            # kv first (every BCD iteration needs the full kT/vT), then q
            for c in range(NCH):
                cs = slice(c * FREE, (c + 1) * FREE)
                xc_t = xpool.tile([P, DT, FREE], F16, tag="xch")
                nc.sync.dma_start(
                    xc_t, xc.rearrange("(i p) t -> p i t", p=P)[:, :, cs]
                )
                pkv = psB.tile([P, FREE], F32, tag="psB")
                for i in range(DT):
                    nc.tensor.matmul(
                        pkv,
                        (wkv_sb[:, i, :]),
                        (xc_t[:, i, :]),
                        start=(i == 0),
                        stop=(i == DT - 1),
                    )
                nc.vector.tensor_copy(kv[:, cs], pkv)
                # duplicate kT rows into partitions 64..127 for row-packing
                nc.sync.dma_start(k2[HEAD_DIM : 2 * HEAD_DIM, cs], kv[:HEAD_DIM, cs])

            def emit_q_chunk(c):
                cs = slice(c * FREE, (c + 1) * FREE)
                xq_t = xpool.tile([P, DT, FREE], F16, tag="xch", name="xq_t")
                nc.sync.dma_start(
                    xq_t, xq.rearrange("(i p) t -> p i t", p=P)[:, :, cs]
                )
                for e in range(ET):
                    pq = psA.tile([P, FREE], F32, tag="psA", name="pq")
                    for i in range(DT):
                        nc.tensor.matmul(
                            pq,
                            (wq_sb[:, i, e * P : (e + 1) * P]),
                            (xq_t[:, i, :]),
                            start=(i == 0),
                            stop=(i == DT - 1),
                        )
                    nc.vector.tensor_copy(qt[:, e, cs], pq)

            for _c in range(min(2, NCH)):
                emit_q_chunk(_c)

            # v' tiles: PE-transpose vT[64, tk*128 ..] -> [128, 64], then
            # build [v | ones] (for even heads) and [ones | v] (odd heads).
            # The all-ones half makes the same matmul emit the softmax
            # denominators, replicated across 64 partitions.
            for t in range(NTK):
                ts_ = slice(t * P, (t + 1) * P)
                pv = psB.tile([P, HEAD_DIM], F16, tag="psB")
                nc.tensor.transpose(
                    pv, kv[HEAD_DIM : 2 * HEAD_DIM, ts_], ident[HEAD_DIM:, HEAD_DIM:P]
                )
                nc.vector.tensor_copy(vp[:, t, :HEAD_DIM], pv)
                nc.vector.tensor_copy(vp2[:, t, HEAD_DIM:], pv)
                nc.vector.tensor_copy(vp[:, t, HEAD_DIM:], ones)
                nc.vector.tensor_copy(vp2[:, t, :HEAD_DIM], ones)

            # -------- stage E chunk emitter (interleaved into BCD stream)
            def emit_out_chunk(c):
                cs = slice(c * FREE, (c + 1) * FREE)
                for m in range(MT):
                    ms = slice(m * P, (m + 1) * P)
                    py = psA.tile([P, FREE], F32, tag="psA", name="py")
                    for ee in range(ET):
                        nc.tensor.matmul(
                            py,
                            (wo_sb[:, ee, ms]),
                            (outs[:, ee, cs]),
                            start=(ee == 0),
                            stop=(ee == ET - 1),
                        )
                    yo = ypool.tile([P, FREE], F32, tag="yout", name="yo")
                    nc.vector.tensor_copy(yo, py)
                    nc.sync.dma_start(yt[ms, cs], yo)

            # ---------------- stages B/C/D: attention per head-pair
            first_bcd = True
            for blk in range(NBLK):
                for e in range(ET):  # head pair (h_even=2e, h_odd=2e+1)
                    bs = slice(blk * BLK, (blk + 1) * BLK)
                    pd = [
                        psB.tile([P, BLK], F32, tag="psB", name=f"pd{_h}")
                        for _h in range(2)
                    ]  # D accumulators: [0]=h_even rows 0-64, [1]=h_odd
                    for t in range(NTK):
                        ts_ = slice(t * P, (t + 1) * P)
                        pb = [
                            psA.tile([P, BLK], F32, tag="psA", name=f"pb{_h}")
                            for _h in range(2)
                        ]
                        for j in range(JPB):
                            js = slice(blk * BLK + j * FREE, blk * BLK + (j + 1) * FREE)
                            jo = slice(j * FREE, (j + 1) * FREE)
                            # scores, 2 heads row-packed (K=64 each)
                            nc.tensor.matmul(
                                pb[0][:, jo],
                                (kv[:HEAD_DIM, ts_]),
                                (qt[:HEAD_DIM, e, js]),
                            )
                            nc.tensor.matmul(
                                pb[1][:, jo],
                                (k2[HEAD_DIM:, ts_]),
                                (qt[HEAD_DIM:, e, js]),
                            )
                        for h in range(2):
                            pt = ptpool.tile([P, BLK], F16, tag="pt")
                            nc.scalar.activation(
                                pt,
                                pb[h],
                                mybir.ActivationFunctionType.Exp,
                                bias=0.0,
                                scale=SCALE,
                            )
                            # M=128 stationary [v|ones] / [ones|v]: one
                            # matmul per head yields out_h in its 64-row
                            # half and the softmax denominators (replicated
                            # x64) in the other half.  dst base stays 0
                            # (fp32r matmuls cannot target offset psum
                            # partitions).
                            vo = vp if h == 0 else vp2
                            for j in range(JPB):
                                jo = slice(j * FREE, (j + 1) * FREE)
                                nc.tensor.matmul(
                                    pd[h][:, jo],
                                    vo[:, t, :],
                                    pt[:, jo],
                                    start=(t == 0),
                                    stop=(t == NTK - 1),
                                    skip_group_check=True,
                                )
                    if first_bcd:
                        first_bcd = False
                        for _c in range(2, NCH):
                            emit_q_chunk(_c)
                    # spill raw accumulators to SBUF immediately (~1.2us)
                    # so the PSUM slots free up and the PE never stalls;
                    # the normalize chain below runs off the critical path.
                    for h in range(2):
                        raw = nrmpool.tile([P, BLK], F32, tag=f"raw{h}")
                        nc.vector.tensor_copy(raw, pd[h])
                        lo = slice(0, 64) if h == 0 else slice(64, 128)
                        hi = slice(64, 128) if h == 0 else slice(0, 64)
                        rec = nrmpool.tile([P, BLK], F32, tag="rec")
                        rec2 = nrmpool.tile([P, BLK], F32, tag="rec2")
                        nc.vector.reciprocal(rec[hi, :], raw[hi, :])
                        nc.sync.dma_start(rec2[lo, :], rec[hi, :])
                        nc.vector.tensor_mul(
                            outs[lo, e, bs], raw[lo, :], rec2[lo, :]
                        )
                    if e == ET - 1:
                        for _c in range(blk * (BLK // FREE), (blk + 1) * (BLK // FREE)):
                            emit_out_chunk(_c)


    nc.finalize()  # Bacc: runs wait-splitting/reg-alloc passes
    return nc


_NC_CACHE = None


def _get_nc():
    global _NC_CACHE
    if _NC_CACHE is None:
        _NC_CACHE = build_bass()
    return _NC_CACHE


def _cid():
    c = np.zeros((P, P + 64), dtype=np.float16)
    c[:, :P] = np.eye(P, dtype=np.float32)
    c[:, P:] = 1.0
    return c


def shard_inputs(query, context, Wq, Wk, Wv, Wo):
    """host-side sharding: 8 cores = batch(2) x kv-group(4)"""
    in_maps = []
    xqT = [np.ascontiguousarray(query[b].T).astype(np.float16) for b in range(B)]
    xcT = [np.ascontiguousarray(context[b].T).astype(np.float16) for b in range(B)]
    for core in range(N_CORES):
        b, g = divmod(core, GROUPS)
        wqT = np.ascontiguousarray(Wq[g * DQ : (g + 1) * DQ, :].T).astype(np.float16)
        wkvT = np.ascontiguousarray(
            np.concatenate(
                [
                    Wk[g * HEAD_DIM : (g + 1) * HEAD_DIM, :],
                    Wv[g * HEAD_DIM : (g + 1) * HEAD_DIM, :],
                ],
                axis=0,
            ).T
        ).astype(np.float16)
        woT = np.ascontiguousarray(Wo[:, g * DQ : (g + 1) * DQ].T).astype(np.float16)
        in_maps.append(
            {
                "xqT": xqT[b],
                "xcT": xcT[b],
                "wqT": wqT,
                "wkvT": wkvT,
                "woT": woT,
                "cid": _cid(),
            }
        )
    return in_maps


def kernel(query, context, Wq, Wk, Wv, Wo, _want_profile=False):
    from concourse.bass_utils import run_bass_kernel_spmd

    nc = _get_nc()
    in_maps = shard_inputs(query, context, Wq, Wk, Wv, Wo)
    res = run_bass_kernel_spmd(
        nc, in_maps, core_ids=list(range(N_CORES)), trace=_want_profile
    )
    out = np.zeros((B, TQ, D_MODEL), dtype=np.float32)
    for core in range(N_CORES):
        b = core // GROUPS
        out[b] += res.results[core]["yT"].T
    if _want_profile:
        return out, res
    return out



# revision 7
# speedup vs baseline: 1.4171x; 1.4171x over previous
"""Cross-attention (GQA) Trainium2 Bass kernel — pipelined v2.

Problem: B=2, Tq=Tkv=2048, D_MODEL=1024, 16 query heads / 4 kv heads,
head_dim=64.  Sharded over 8 NeuronCores as batch(2) x kv-group(4); each
core computes 4 query heads + its single kv head and a partial output
projection (Wo row-split by head group); partials are summed on host.

Dataflow (feature dim on SBUF partitions end-to-end, no big transposes):

  A: qT[e,t] = WqT.T @ xqT,  kvT = WkvT.T @ xcT   (weights stationary)
     v[tk,dv] via PE-transpose of vT tiles; vp=[v|1], vp2=[1|v]
  B: per (blk,e) section, unit t: pb[128,1024] = two K=64 row-group
     matmuls (h_even rows 0-63 -> cols 0:512, h_odd rows 64-127 ->
     cols 512:1024), concurrent in the PE array.
  C: pt = exp(pb/8) one ScalarE instruction per unit (FD=1024).
  D: pd_h[128,512] += vp_t.T @ pt_half; ones-columns give the softmax
     denominators in the complementary 64 partitions.
  E: yT += WoT.T @ (pd*recip(den)), row-split by head pair.

The whole BCD stream is software-pipelined: the PE emission order is
B(t), D(t-1) so matmuls never wait on the ScalarE exp of the same unit;
projection/output-projection matmuls are fed as "fill" work into the
PE slack inside each section.  ScalarE (the 1 elem/cycle/lane exp
bottleneck, ~143us) paces the kernel; the PE stays dense and HAM-warm.
"""

import os
import sys
from collections import deque

import numpy as np

for _p in ("/opt/trn_rl_repo",):
    if _p not in sys.path and os.path.isdir(_p):
        sys.path.insert(0, _p)

import concourse.bass as bass
import concourse.bacc as bacc
import concourse.mybir as mybir
from concourse.tile import TileContext

# ---------------------------------------------------------------- problem dims
B = 2
TQ = 2048
TKV = 2048
D_MODEL = 1024
N_HEADS = 16
N_KV_HEADS = 4
HEAD_DIM = 64
N_CORES = 8
GROUPS = N_KV_HEADS  # kv groups = 4
HEADS_PER_DEV = N_HEADS // GROUPS  # 4
DQ = HEADS_PER_DEV * HEAD_DIM  # 256
DKV = 2 * HEAD_DIM  # 128 (k rows + v rows stacked)
SCALE = 1.0 / float(np.sqrt(HEAD_DIM))

P = 128
FREE = 512  # matmul moving-operand chunk / tq block width
BLK = 512
NBLK = TQ // BLK  # 4 tq blocks
DT = D_MODEL // P  # 8 d-tiles
ET = DQ // P  # 2 e-tiles (query head pairs)
NCH = TQ // FREE  # 4 x chunks of 512
NTK = TKV // P  # 16 tk tiles
MT = D_MODEL // P  # 8 output m-tiles

F32 = mybir.dt.float32
F16 = mybir.dt.float16


def build_bass():
    nc = bacc.Bacc()

    xq = nc.declare_dram_parameter("xqT", [D_MODEL, TQ], F16, isOutput=False)
    xc = nc.declare_dram_parameter("xcT", [D_MODEL, TKV], F16, isOutput=False)
    wq = nc.declare_dram_parameter("wqT", [D_MODEL, DQ], F16, isOutput=False)
    wkv = nc.declare_dram_parameter("wkvT", [D_MODEL, DKV], F16, isOutput=False)
    wo = nc.declare_dram_parameter("woT", [DQ, D_MODEL], F16, isOutput=False)
    cid = nc.declare_dram_parameter("cid", [P, P], F16, isOutput=False)
    yt = nc.declare_dram_parameter("yT", [D_MODEL, TQ], F16, isOutput=True)

    with TileContext(nc) as tc:
        with (
            tc.tile_pool(name="consts", bufs=1) as consts,
            tc.tile_pool(name="xch", bufs=2) as xpool,
            tc.tile_pool(name="pt", bufs=3) as ptpool,
            tc.tile_pool(name="rec", bufs=2) as recpool,
            tc.tile_pool(name="yout", bufs=3) as ypool,
            tc.tile_pool(name="psS", bufs=2, space="PSUM") as psS,
            tc.tile_pool(name="psD", bufs=1, space="PSUM") as psD,
            tc.tile_pool(name="psA", bufs=2, space="PSUM") as psA,
        ):
            # ---------------- constants / persistent tiles
            ident = consts.tile([P, P], F16, tag="ident")
            nc.sync.dma_start(ident, cid[:])

            wkv_sb = consts.tile([P, DT, DKV], F16, tag="wkv")
            nc.sync.dma_start(wkv_sb, wkv.rearrange("(i p) e -> p i e", p=P))
            wq_sb = consts.tile([P, DT, DQ], F16, tag="wq")
            nc.sync.dma_start(wq_sb, wq.rearrange("(i p) e -> p i e", p=P))
            wo_sb = consts.tile([P, ET, D_MODEL], F16, tag="wo")
            nc.sync.dma_start(wo_sb, wo.rearrange("(i p) m -> p i m", p=P))

            qt = consts.tile([P, ET, TQ], F16, tag="qt")  # head pair per e
            kv = consts.tile([P, TKV], F16, tag="kv")  # rows 0-63 kT, 64-127 vT
            k2 = consts.tile([P, TKV], F16, tag="k2")  # rows 64-127 = kT copy
            vp = consts.tile([P, NTK, P], F16, tag="vp")  # [v | ones]
            vp2 = consts.tile([P, NTK, P], F16, tag="vp2")  # [ones | v]
            outs = consts.tile([P, ET, TQ], F16, tag="outs")  # normalized outT

            nc.vector.memset(vp, 1.0)
            nc.vector.memset(vp2, 1.0)

            # input chunk dmas (xpool rotates 2 bufs per tag)
            def dma_xc(c):
                cs = slice(c * FREE, (c + 1) * FREE)
                t = xpool.tile([P, DT, FREE], F16, tag="xc", name=f"xc{c}")
                nc.sync.dma_start(t, xc.rearrange("(i p) t -> p i t", p=P)[:, :, cs])
                return t

            def dma_xq(c):
                cs = slice(c * FREE, (c + 1) * FREE)
                t = xpool.tile([P, DT, FREE], F16, tag="xq", name=f"xq{c}")
                nc.sync.dma_start(t, xq.rearrange("(i p) t -> p i t", p=P)[:, :, cs])
                return t

            # ---------------- fill-work machinery (PE slack consumers)
            fills = deque()

            def pop_fill(n=1):
                for _ in range(n):
                    if not fills:
                        return
                    fills.popleft()()

            # D matmuls for one pipelined unit (two heads, K=128, N=512)
            def emit_d(pd0, pd1, pt, t):
                nc.tensor.matmul(
                    pd0, vp[:, t, :], pt[:, :BLK],
                    start=(t == 0), stop=(t == NTK - 1), skip_group_check=True,
                )
                nc.tensor.matmul(
                    pd1, vp2[:, t, :], pt[:, BLK:],
                    start=(t == 0), stop=(t == NTK - 1), skip_group_check=True,
                )

            # kv projection chunk: 8 K-tiles -> kv[:, cs]; k2 copy; transposes
            def kv_chunk_pieces(c, get_xc):
                cs = slice(c * FREE, (c + 1) * FREE)
                st = {}

                def p1():
                    st["pkv"] = psA.tile([P, FREE], F32, tag="pa", name="pkv")
                    for i in range(4):
                        nc.tensor.matmul(
                            st["pkv"], wkv_sb[:, i, :], get_xc()[:, i, :],
                            start=(i == 0), stop=False,
                        )

                def p2():
                    for i in range(4, DT):
                        nc.tensor.matmul(
                            st["pkv"], wkv_sb[:, i, :], get_xc()[:, i, :],
                            start=False, stop=(i == DT - 1),
                        )
                    nc.vector.tensor_copy(kv[:, cs], st["pkv"])
                    nc.sync.dma_start(k2[HEAD_DIM:, cs], kv[:HEAD_DIM, cs])

                def p3():
                    # transpose the 4 v tiles of this chunk, batch-copy to vp/vp2
                    pvb = psA.tile([P, 4 * HEAD_DIM], F16, tag="pa", name="pvb")
                    for k in range(4):
                        ts_ = slice((4 * c + k) * P, (4 * c + k + 1) * P)
                        nc.tensor.transpose(
                            pvb[:, k * HEAD_DIM : (k + 1) * HEAD_DIM],
                            kv[HEAD_DIM:, ts_],
                            ident[HEAD_DIM:, HEAD_DIM:],
                        )
                    src = pvb.rearrange("p (k d) -> p k d", k=4)
                    nc.vector.tensor_copy(vp[:, 4 * c : 4 * c + 4, :HEAD_DIM], src)
                    nc.vector.tensor_copy(vp2[:, 4 * c : 4 * c + 4, HEAD_DIM:], src)

                return [p1, p2, p3]

            # q projection chunk (one e): 8 K-tiles -> qt[:, e, cs]
            def q_chunk_pieces(c, e, get_xq):
                cs = slice(c * FREE, (c + 1) * FREE)
                st = {}

                def p1():
                    st["pq"] = psA.tile([P, FREE], F32, tag="pa", name="pq")
                    for i in range(4):
                        nc.tensor.matmul(
                            st["pq"], wq_sb[:, i, e * P : (e + 1) * P], get_xq()[:, i, :],
                            start=(i == 0), stop=False,
                        )

                def p2():
                    for i in range(4, DT):
                        nc.tensor.matmul(
                            st["pq"], wq_sb[:, i, e * P : (e + 1) * P], get_xq()[:, i, :],
                            start=False, stop=(i == DT - 1),
                        )
                    nc.vector.tensor_copy(qt[:, e, cs], st["pq"])

                return [p1, p2]

            # output-projection piece for one m-tile of one tq block
            def e_piece(blk, m):
                bs = slice(blk * BLK, (blk + 1) * BLK)
                ms = slice(m * P, (m + 1) * P)

                def p():
                    py = psA.tile([P, FREE], F32, tag="pa", name="py")
                    for ee in range(ET):
                        nc.tensor.matmul(
                            py, wo_sb[:, ee, ms], outs[:, ee, bs],
                            start=(ee == 0), stop=(ee == ET - 1),
                        )
                    yo = ypool.tile([P, FREE], F16, tag="yo", name="yo")
                    nc.vector.tensor_copy(yo, py)
                    nc.sync.dma_start(yt[ms, bs], yo)

                return p

            # ---------------- lead-in: weights, first chunks, kv c0, q c0
            xc_t = [None] * NCH
            xq_t = [None] * NCH
            xc_t[0] = dma_xc(0)
            xq_t[0] = dma_xq(0)

            for piece in kv_chunk_pieces(0, lambda: xc_t[0]):
                piece()
            for e in range(ET):
                for piece in q_chunk_pieces(0, e, lambda: xq_t[0]):
                    piece()

            # remaining input dmas + projections go through the fill queue
            xc_t[1] = dma_xc(1)
            for c in (1, 2, 3):
                if c >= 2:
                    fills.append(lambda c=c: xc_t.__setitem__(c, dma_xc(c)))
                fills.extend(kv_chunk_pieces(c, lambda c=c: xc_t[c]))
            fills.append(lambda: xq_t.__setitem__(1, dma_xq(1)))
            for e in range(ET):
                fills.extend(q_chunk_pieces(1, e, lambda: xq_t[1]))

            # ---------------- BCD sections
            for sec, (blk, e) in enumerate(
                (blk, e) for blk in range(NBLK) for e in range(ET)
            ):
                bs = slice(blk * BLK, (blk + 1) * BLK)
                pd0 = psD.tile([P, BLK], F32, tag="pd0", name="pd0")
                pd1 = psD.tile([P, BLK], F32, tag="pd1", name="pd1")
                prev = None
                for t in range(NTK):
                    ts_ = slice(t * P, (t + 1) * P)
                    pb = psS.tile([P, 2 * BLK], F32, tag="pb", name="pb")
                    # B: two K=64 row-group matmuls, concurrent in the array
                    nc.tensor.matmul(pb[:, :BLK], kv[:HEAD_DIM, ts_], qt[:HEAD_DIM, e, bs])
                    nc.tensor.matmul(pb[:, BLK:], k2[HEAD_DIM:, ts_], qt[HEAD_DIM:, e, bs])
                    if prev is not None:
                        emit_d(*prev)
                    pt = ptpool.tile([P, 2 * BLK], F16, tag="pt", name="pt")
                    nc.scalar.activation(
                        pt, pb, mybir.ActivationFunctionType.Exp, bias=0.0, scale=SCALE
                    )
                    pop_fill(2 if sec == 0 else 1)
                    prev = (pd0, pd1, pt, t)
                emit_d(*prev)

                # normalize: recip(den) -> broadcast -> outs = out * rec
                rec0 = recpool.tile([P, BLK], F32, tag="rec0", name="rec0")
                rec1 = recpool.tile([P, BLK], F32, tag="rec1", name="rec1")
                nc.vector.reciprocal(rec0[HEAD_DIM:, :], pd0[HEAD_DIM:, :])
                nc.vector.reciprocal(rec1[:HEAD_DIM, :], pd1[:HEAD_DIM, :])
                nc.sync.dma_start(rec0[:HEAD_DIM, :], rec0[HEAD_DIM:, :])
                nc.sync.dma_start(rec1[HEAD_DIM:, :], rec1[:HEAD_DIM, :])
                nc.vector.tensor_mul(
                    outs[:HEAD_DIM, e, bs], pd0[:HEAD_DIM, :], rec0[:HEAD_DIM, :]
                )
                nc.vector.tensor_mul(
                    outs[HEAD_DIM:, e, bs], pd1[HEAD_DIM:, :], rec1[HEAD_DIM:, :]
                )

                # queue follow-on work
                if e == ET - 1:
                    for m in range(MT):
                        fills.append(e_piece(blk, m))
                if sec == 1:
                    fills.append(lambda: xq_t.__setitem__(2, dma_xq(2)))
                    for ee in range(ET):
                        fills.extend(q_chunk_pieces(2, ee, lambda: xq_t[2]))
                if sec == 3:
                    fills.append(lambda: xq_t.__setitem__(3, dma_xq(3)))
                    for ee in range(ET):
                        fills.extend(q_chunk_pieces(3, ee, lambda: xq_t[3]))

            # tail: drain remaining fill work (last block's output projection)
            while fills:
                pop_fill()

    nc.finalize()
    return nc


_NC_CACHE = None


def _get_nc():
    global _NC_CACHE
    if _NC_CACHE is None:
        _NC_CACHE = build_bass()
    return _NC_CACHE


def _cid():
    return np.eye(P, dtype=np.float16)


def shard_inputs(query, context, Wq, Wk, Wv, Wo):
    """host-side sharding: 8 cores = batch(2) x kv-group(4)"""
    in_maps = []
    xqT = [np.ascontiguousarray(query[b].T).astype(np.float16) for b in range(B)]
    xcT = [np.ascontiguousarray(context[b].T).astype(np.float16) for b in range(B)]
    for core in range(N_CORES):
        b, g = divmod(core, GROUPS)
        wqT = np.ascontiguousarray(Wq[g * DQ : (g + 1) * DQ, :].T).astype(np.float16)
        wkvT = np.ascontiguousarray(
            np.concatenate(
                [
                    Wk[g * HEAD_DIM : (g + 1) * HEAD_DIM, :],
                    Wv[g * HEAD_DIM : (g + 1) * HEAD_DIM, :],
                ],
                axis=0,
            ).T
        ).astype(np.float16)
        woT = np.ascontiguousarray(Wo[:, g * DQ : (g + 1) * DQ].T).astype(np.float16)
        in_maps.append(
            {
                "xqT": xqT[b],
                "xcT": xcT[b],
                "wqT": wqT,
                "wkvT": wkvT,
                "woT": woT,
                "cid": _cid(),
            }
        )
    return in_maps


def kernel(query, context, Wq, Wk, Wv, Wo, _want_profile=False):
    from concourse.bass_utils import run_bass_kernel_spmd

    nc = _get_nc()
    in_maps = shard_inputs(query, context, Wq, Wk, Wv, Wo)
    res = run_bass_kernel_spmd(
        nc, in_maps, core_ids=list(range(N_CORES)), trace=_want_profile
    )
    out = np.zeros((B, TQ, D_MODEL), dtype=np.float32)
    for core in range(N_CORES):
        b = core // GROUPS
        out[b] += res.results[core]["yT"].T.astype(np.float32)
    if _want_profile:
        return out, res
    return out


# revision 13
# speedup vs baseline: 1.5361x; 1.0840x over previous
"""Cross-attention (GQA) Trainium2 Bass kernel — pipelined v2.

Problem: B=2, Tq=Tkv=2048, D_MODEL=1024, 16 query heads / 4 kv heads,
head_dim=64.  Sharded over 8 NeuronCores as batch(2) x kv-group(4); each
core computes 4 query heads + its single kv head and a partial output
projection (Wo row-split by head group); partials are summed on host.

Dataflow (feature dim on SBUF partitions end-to-end, no big transposes):

  A: qT[e,t] = WqT.T @ xqT,  kvT = WkvT.T @ xcT   (weights stationary)
     v[tk,dv] via PE-transpose of vT tiles; vp=[v|1], vp2=[1|v]
  B: per (blk,e) section, unit t: pb[128,1024] = two K=64 row-group
     matmuls (h_even rows 0-63 -> cols 0:512, h_odd rows 64-127 ->
     cols 512:1024), concurrent in the PE array.
  C: pt = exp(pb/8) one ScalarE instruction per unit (FD=1024).
  D: pd_h[128,512] += vp_t.T @ pt_half; ones-columns give the softmax
     denominators in the complementary 64 partitions.
  E: yT += WoT.T @ (pd*recip(den)), row-split by head pair.

The whole BCD stream is software-pipelined: the PE emission order is
B(t), D(t-1) so matmuls never wait on the ScalarE exp of the same unit;
projection/output-projection matmuls are fed as "fill" work into the
PE slack inside each section.  ScalarE (the 1 elem/cycle/lane exp
bottleneck, ~143us) paces the kernel; the PE stays dense and HAM-warm.
"""

import os
import sys
from collections import deque

import numpy as np

for _p in ("/opt/trn_rl_repo",):
    if _p not in sys.path and os.path.isdir(_p):
        sys.path.insert(0, _p)

import concourse.bass as bass
import concourse.bacc as bacc
import concourse.mybir as mybir
from concourse.tile import TileContext

# ---------------------------------------------------------------- problem dims
B = 2
TQ = 2048
TKV = 2048
D_MODEL = 1024
N_HEADS = 16
N_KV_HEADS = 4
HEAD_DIM = 64
N_CORES = 8
GROUPS = N_KV_HEADS  # kv groups = 4
HEADS_PER_DEV = N_HEADS // GROUPS  # 4
DQ = HEADS_PER_DEV * HEAD_DIM  # 256
DKV = 2 * HEAD_DIM  # 128 (k rows + v rows stacked)
SCALE = 1.0 / float(np.sqrt(HEAD_DIM))

P = 128
FREE = 512  # matmul moving-operand chunk / tq block width
BLK = 512
NBLK = TQ // BLK  # 4 tq blocks
DT = D_MODEL // P  # 8 d-tiles
ET = DQ // P  # 2 e-tiles (query head pairs)
NCH = TQ // FREE  # 4 x chunks of 512
NTK = TKV // P  # 16 tk tiles
MT = D_MODEL // P  # 8 output m-tiles

F32 = mybir.dt.float32
F16 = mybir.dt.float16


def build_bass():
    nc = bacc.Bacc()

    xq = nc.declare_dram_parameter("xqT", [D_MODEL, TQ], F16, isOutput=False)
    xc = nc.declare_dram_parameter("xcT", [D_MODEL, TKV], F16, isOutput=False)
    wq = nc.declare_dram_parameter("wqT", [D_MODEL, DQ], F16, isOutput=False)
    wkv = nc.declare_dram_parameter("wkvT", [D_MODEL, DKV], F16, isOutput=False)
    wo = nc.declare_dram_parameter("woT", [DQ, D_MODEL], F16, isOutput=False)
    cid = nc.declare_dram_parameter("cid", [P, P], F16, isOutput=False)
    yt = nc.declare_dram_parameter("yT", [D_MODEL, TQ], F16, isOutput=True)

    with TileContext(nc) as tc:
        with (
            tc.tile_pool(name="consts", bufs=1) as consts,
            tc.tile_pool(name="xch", bufs=2) as xpool,
            tc.tile_pool(name="pt", bufs=3) as ptpool,
            tc.tile_pool(name="rec", bufs=2) as recpool,
            tc.tile_pool(name="yout", bufs=3) as ypool,
            tc.tile_pool(name="psS", bufs=2, space="PSUM") as psS,
            tc.tile_pool(name="psD", bufs=1, space="PSUM") as psD,
            tc.tile_pool(name="psA", bufs=2, space="PSUM") as psA,
        ):
            # ---------------- constants / persistent tiles
            # DMA priority order: the lead-in critical path is
            # xc0+wkv (kv proj) then xq0+wq (q proj) -> first B matmul.
            qt = consts.tile([P, ET, TQ], F16, tag="qt")  # head pair per e
            kv = consts.tile([P, TKV], F16, tag="kv")  # rows 0-63 kT, 64-127 vT
            k2 = consts.tile([P, TKV], F16, tag="k2")  # rows 64-127 = kT copy
            vp = consts.tile([P, NTK, P], F16, tag="vp")  # [v | ones]
            vp2 = consts.tile([P, NTK, P], F16, tag="vp2")  # [ones | v]
            outs = consts.tile([P, ET, TQ], F16, tag="outs")  # normalized outT

            # input chunk dmas (xpool rotates 2 bufs per tag)
            def dma_xc(c):
                cs = slice(c * FREE, (c + 1) * FREE)
                t = xpool.tile([P, DT, FREE], F16, tag="xc", name=f"xc{c}")
                nc.sync.dma_start(t, xc.rearrange("(i p) t -> p i t", p=P)[:, :, cs])
                return t

            def dma_xq(c):
                cs = slice(c * FREE, (c + 1) * FREE)
                t = xpool.tile([P, DT, FREE], F16, tag="xq", name=f"xq{c}")
                nc.sync.dma_start(t, xq.rearrange("(i p) t -> p i t", p=P)[:, :, cs])
                return t

            xc_t = [None] * NCH
            xq_t = [None] * NCH
            xc_t[0] = dma_xc(0)
            wkv_sb = consts.tile([P, DT, DKV], F16, tag="wkv")
            nc.sync.dma_start(wkv_sb, wkv.rearrange("(i p) e -> p i e", p=P))
            xq_t[0] = dma_xq(0)
            wq_sb = consts.tile([P, DT, DQ], F16, tag="wq")
            nc.sync.dma_start(wq_sb, wq.rearrange("(i p) e -> p i e", p=P))
            ident = consts.tile([P, P], F16, tag="ident")
            nc.sync.dma_start(ident, cid[:])
            wo_sb = consts.tile([P, ET, D_MODEL], F16, tag="wo")

            nc.vector.memset(vp, 1.0)
            nc.vector.memset(vp2, 1.0)

            # ---------------- fill-work machinery (PE slack consumers)
            fills = deque()

            def pop_fill(n=1):
                for _ in range(n):
                    if not fills:
                        return
                    fills.popleft()()

            # D matmuls for one pipelined unit (two heads, K=128, N=512)
            def emit_d(pd0, pd1, pt, t):
                nc.tensor.matmul(
                    pd0, vp[:, t, :], pt[:, :BLK],
                    start=(t == 0), stop=(t == NTK - 1), skip_group_check=True,
                )
                nc.tensor.matmul(
                    pd1, vp2[:, t, :], pt[:, BLK:],
                    start=(t == 0), stop=(t == NTK - 1), skip_group_check=True,
                )

            # kv projection chunk: 8 K-tiles -> kv[:, cs]; k2 copy; transposes
            def kv_chunk_pieces(c, get_xc):
                cs = slice(c * FREE, (c + 1) * FREE)
                st = {}

                def pk(i0):
                    def p():
                        if i0 == 0:
                            st["pkv"] = psA.tile([P, FREE], F32, tag="pa", name="pkv")
                        for i in range(i0, i0 + 2):
                            nc.tensor.matmul(
                                st["pkv"], wkv_sb[:, i, :], get_xc()[:, i, :],
                                start=(i == 0), stop=(i == DT - 1),
                            )
                        if i0 == DT - 2:
                            nc.vector.tensor_copy(kv[:, cs], st["pkv"])
                            nc.sync.dma_start(k2[HEAD_DIM:, cs], kv[:HEAD_DIM, cs])

                    return p

                def p3():
                    # transpose the 4 v tiles of this chunk, batch-copy to vp/vp2
                    pvb = psA.tile([P, 4 * HEAD_DIM], F16, tag="pa", name="pvb")
                    for k in range(4):
                        ts_ = slice((4 * c + k) * P, (4 * c + k + 1) * P)
                        nc.tensor.transpose(
                            pvb[:, k * HEAD_DIM : (k + 1) * HEAD_DIM],
                            kv[HEAD_DIM:, ts_],
                            ident[HEAD_DIM:, HEAD_DIM:],
                        )
                    src = pvb.rearrange("p (k d) -> p k d", k=4)
                    nc.vector.tensor_copy(vp[:, 4 * c : 4 * c + 4, :HEAD_DIM], src)
                    nc.vector.tensor_copy(vp2[:, 4 * c : 4 * c + 4, HEAD_DIM:], src)

                return [pk(0), pk(2), pk(4), pk(6), p3]

            # q projection chunk (one e): 8 K-tiles -> qt[:, e, cs]
            def q_chunk_pieces(c, e, get_xq):
                cs = slice(c * FREE, (c + 1) * FREE)
                st = {}

                def pq(i0):
                    def p():
                        if i0 == 0:
                            st["pq"] = psA.tile([P, FREE], F32, tag="pa", name="pq")
                        for i in range(i0, i0 + 2):
                            nc.tensor.matmul(
                                st["pq"], wq_sb[:, i, e * P : (e + 1) * P],
                                get_xq()[:, i, :],
                                start=(i == 0), stop=(i == DT - 1),
                            )
                        if i0 == DT - 2:
                            nc.vector.tensor_copy(qt[:, e, cs], st["pq"])

                    return p

                return [pq(0), pq(2), pq(4), pq(6)]

            # output-projection piece for one m-tile of one tq block
            def e_piece(blk, m):
                bs = slice(blk * BLK, (blk + 1) * BLK)
                ms = slice(m * P, (m + 1) * P)

                def p():
                    py = psA.tile([P, FREE], F32, tag="pa", name="py")
                    for ee in range(ET):
                        nc.tensor.matmul(
                            py, wo_sb[:, ee, ms], outs[:, ee, bs],
                            start=(ee == 0), stop=(ee == ET - 1),
                        )
                    yo = ypool.tile([P, FREE], F16, tag="yo", name="yo")
                    nc.vector.tensor_copy(yo, py)
                    nc.sync.dma_start(yt[ms, bs], yo)

                return p

            # ---------------- lead-in: kv c0 + q c0 inline, rest as fills
            for piece in kv_chunk_pieces(0, lambda: xc_t[0]):
                piece()
            for e in range(ET):
                for piece in q_chunk_pieces(0, e, lambda: xq_t[0]):
                    piece()

            # remaining input dmas + projections go through the fill queue
            xc_t[1] = dma_xc(1)
            fills.append(
                lambda: nc.sync.dma_start(wo_sb, wo.rearrange("(i p) m -> p i m", p=P))
            )
            for c in (1, 2, 3):
                if c >= 2:
                    fills.append(lambda c=c: xc_t.__setitem__(c, dma_xc(c)))
                fills.extend(kv_chunk_pieces(c, lambda c=c: xc_t[c]))
            fills.append(lambda: xq_t.__setitem__(1, dma_xq(1)))
            for e in range(ET):
                fills.extend(q_chunk_pieces(1, e, lambda: xq_t[1]))

            # ---------------- BCD sections
            for sec, (blk, e) in enumerate(
                (blk, e) for blk in range(NBLK) for e in range(ET)
            ):
                bs = slice(blk * BLK, (blk + 1) * BLK)
                pd0 = psD.tile([P, BLK], F32, tag="pd0", name="pd0")
                pd1 = psD.tile([P, BLK], F32, tag="pd1", name="pd1")
                prev = None
                for t in range(NTK):
                    ts_ = slice(t * P, (t + 1) * P)
                    pb = psS.tile([P, 2 * BLK], F32, tag="pb", name="pb")
                    # B: two K=64 row-group matmuls, concurrent in the array
                    nc.tensor.matmul(pb[:, :BLK], kv[:HEAD_DIM, ts_], qt[:HEAD_DIM, e, bs])
                    nc.tensor.matmul(pb[:, BLK:], k2[HEAD_DIM:, ts_], qt[HEAD_DIM:, e, bs])
                    if prev is not None:
                        emit_d(*prev)
                    pt = ptpool.tile([P, 2 * BLK], F16, tag="pt", name="pt")
                    nc.scalar.activation(
                        pt, pb, mybir.ActivationFunctionType.Exp, bias=0.0, scale=SCALE
                    )
                    pop_fill(2 if sec == 0 else 1)
                    prev = (pd0, pd1, pt, t)
                emit_d(*prev)

                # normalize: spill pd fast (frees PSUM for the next section),
                # then recip(den) -> broadcast -> outs = out * rec off-path.
                raw0 = recpool.tile([P, BLK], F32, tag="raw0", name="raw0")
                raw1 = recpool.tile([P, BLK], F32, tag="raw1", name="raw1")
                nc.vector.tensor_copy(raw0, pd0)
                nc.vector.tensor_copy(raw1, pd1)
                rec0 = recpool.tile([P, BLK], F32, tag="rec0", name="rec0")
                rec1 = recpool.tile([P, BLK], F32, tag="rec1", name="rec1")
                # last section: half-width chain halves the tail latency
                halves = 2 if sec == NBLK * ET - 1 else 1
                hw = BLK // halves
                for h in range(halves):
                    hs = slice(h * hw, (h + 1) * hw)
                    bh = slice(blk * BLK + h * hw, blk * BLK + (h + 1) * hw)
                    nc.vector.reciprocal(rec0[HEAD_DIM:, hs], raw0[HEAD_DIM:, hs])
                    nc.sync.dma_start(rec0[:HEAD_DIM, hs], rec0[HEAD_DIM:, hs])
                    nc.vector.reciprocal(rec1[:HEAD_DIM, hs], raw1[:HEAD_DIM, hs])
                    nc.vector.tensor_mul(
                        outs[:HEAD_DIM, e, bh], raw0[:HEAD_DIM, hs], rec0[:HEAD_DIM, hs]
                    )
                    nc.sync.dma_start(rec1[HEAD_DIM:, hs], rec1[:HEAD_DIM, hs])
                    nc.vector.tensor_mul(
                        outs[HEAD_DIM:, e, bh], raw1[HEAD_DIM:, hs], rec1[HEAD_DIM:, hs]
                    )

                # queue follow-on work
                if e == ET - 1:
                    for m in range(MT):
                        fills.append(e_piece(blk, m))
                if sec == 1:
                    fills.append(lambda: xq_t.__setitem__(2, dma_xq(2)))
                    for ee in range(ET):
                        fills.extend(q_chunk_pieces(2, ee, lambda: xq_t[2]))
                if sec == 3:
                    fills.append(lambda: xq_t.__setitem__(3, dma_xq(3)))
                    for ee in range(ET):
                        fills.extend(q_chunk_pieces(3, ee, lambda: xq_t[3]))

            # tail: drain remaining fill work (last block's output projection)
            while fills:
                pop_fill()

    nc.finalize()
    return nc


_NC_CACHE = None


def _get_nc():
    global _NC_CACHE
    if _NC_CACHE is None:
        _NC_CACHE = build_bass()
    return _NC_CACHE


def _cid():
    return np.eye(P, dtype=np.float16)


def shard_inputs(query, context, Wq, Wk, Wv, Wo):
    """host-side sharding: 8 cores = batch(2) x kv-group(4)"""
    in_maps = []
    xqT = [np.ascontiguousarray(query[b].T).astype(np.float16) for b in range(B)]
    xcT = [np.ascontiguousarray(context[b].T).astype(np.float16) for b in range(B)]
    for core in range(N_CORES):
        b, g = divmod(core, GROUPS)
        wqT = np.ascontiguousarray(Wq[g * DQ : (g + 1) * DQ, :].T).astype(np.float16)
        wkvT = np.ascontiguousarray(
            np.concatenate(
                [
                    Wk[g * HEAD_DIM : (g + 1) * HEAD_DIM, :],
                    Wv[g * HEAD_DIM : (g + 1) * HEAD_DIM, :],
                ],
                axis=0,
            ).T
        ).astype(np.float16)
        woT = np.ascontiguousarray(Wo[:, g * DQ : (g + 1) * DQ].T).astype(np.float16)
        in_maps.append(
            {
                "xqT": xqT[b],
                "xcT": xcT[b],
                "wqT": wqT,
                "wkvT": wkvT,
                "woT": woT,
                "cid": _cid(),
            }
        )
    return in_maps


def kernel(query, context, Wq, Wk, Wv, Wo, _want_profile=False):
    from concourse.bass_utils import run_bass_kernel_spmd

    nc = _get_nc()
    in_maps = shard_inputs(query, context, Wq, Wk, Wv, Wo)
    res = run_bass_kernel_spmd(
        nc, in_maps, core_ids=list(range(N_CORES)), trace=_want_profile
    )
    out = np.zeros((B, TQ, D_MODEL), dtype=np.float32)
    for core in range(N_CORES):
        b = core // GROUPS
        out[b] += res.results[core]["yT"].T.astype(np.float32)
    if _want_profile:
        return out, res
    return out
